# revision 47
# baseline (speedup 1.0000x reference)
"""GCN2 (8-layer, N=100K, E=1.6M, G=128) Trainium2 Bass kernel, 8-core SPMD.

Strategy (data-parallel over graphs, per sharding hint):
- batch is sorted => graphs are block-contiguous in node ids. 16 graphs/core.
- Per layer: each core computes h for its node shard; shards are AllGathered
  into a full fp16 table [8*NP, 256] in Shared DRAM; the edge aggregation
  A_hat @ h is done per-core over edges grouped by destination:
    * edges sorted into (dst-block of 128 nodes) x (src-range group of <=2*NP
      rows, so gather indices fit int16), padded to chunks of 128 edges
    * dma_gather pulls 128 rows (512B fp16 each) per chunk from the table
    * a one-hot matmul (lhsT = onehot[edge_slot, dst_local], rhs = gathered
      rows) segment-sums each chunk into the block's PSUM accumulator
  The symmetric gcn_norm is folded in: table rows are pre-scaled by
  dinv[src]; dinv[dst]*(1-alpha) is applied per-partition when copying the
  PSUM accumulator out.
- GCN2Conv update: out = a @ W1' + h0 @ W2'' with W1' = b*W1 + (1-b)*I,
  W2'' = ALPHA*(b*W2 + (1-b)*I) folded on host. h0^T is kept resident in
  SBUF (fp16) as the stationary operand; a is transposed on the PE.
- LayerNorm via bn_stats/bn_aggr (fp32), then gamma/beta + ReLU.
- Mean-pool folded into a per-block one-hot matmul on the last layer
  (weights 1/cnt), then a tiny per-core MLP head -> y[16] per core.

The instruction stream must be identical on all 8 cores (SPMD): per-(block,
group) chunk counts are the elementwise MAX across cores; shorter cores pad
with duplicate gather indices and dst_local=-1 (one-hot row of zeros).
"""

import os
import sys
import types
import numpy as np
from contextlib import ExitStack

sys.path.insert(0, "/opt/trn_rl_repo")
if os.path.isdir("/root/.axon_site"):
    sys.path.insert(0, "/root/.axon_site")

import concourse.bass as bass
import concourse.bacc as bacc
import concourse.tile as tile
from concourse import mybir
from concourse.bass_utils import run_bass_kernel_spmd
from concourse.masks import make_identity

f16 = mybir.dt.float16
f32 = mybir.dt.float32
f8 = mybir.dt.float8e4
i16 = mybir.dt.int16

# problem constants
N, E, G = 100000, 1600000, 128
D_IN, D_H, L = 771, 256, 8
ALPHA, THETA = 0.1, 0.5
LN_EPS = 1e-5
NCORES = 8
GPC = G // NCORES          # graphs per core
NGROUP = 4                 # src-range groups (int16 gather index limit)

LAST_EXEC_NS = None


def _maybe_register_ntff_hook():
    """Register the axon NTFF profiling hook if the image's antenv lacks it."""
    try:
        from antenv import axon_hooks  # noqa: F401
        return
    except ImportError:
        pass
    try:
        from trn_agent_boot.trn_boot import _ntff_profile_via_ctypes
        import antenv
        mod = types.ModuleType("antenv.axon_hooks")
        hook = _ntff_profile_via_ctypes("/opt/axon/libaxon_pjrt.so")
        if hook is None:
            return
        mod.get_axon_ntff_profile_hook = lambda: hook
        sys.modules["antenv.axon_hooks"] = mod
        antenv.axon_hooks = mod
    except Exception:
        pass


# ---------------------------------------------------------------------------
# Host preprocessing
# ---------------------------------------------------------------------------

def preprocess(x, edge_index, batch, lin_in_w, lin_in_b, w1, w2, ln_g, ln_b,
               c1_w, c1_b, c2_w, c2_b, c3_w, c3_b, L_layers=L):
    x = np.asarray(x, np.float32)
    edge_index = np.asarray(edge_index)
    batch = np.asarray(batch)
    n = x.shape[0]
    d_in = x.shape[1]

    # degrees include the gcn_norm self-loop; self-loop edges (added + any
    # natural src==dst edges) are handled by a local diagonal term on-device,
    # so only src!=dst edges enter the gather streams.
    e_src = edge_index[0].astype(np.int64)
    e_dst = edge_index[1].astype(np.int64)
    deg = (np.bincount(e_dst, minlength=n) + 1).astype(np.float64)
    dinv = (1.0 / np.sqrt(deg)).astype(np.float32)  # deg >= 1 via self-loop
    nonself = e_src != e_dst
    src = e_src[nonself]
    dst = e_dst[nonself]
    selfcnt = (1 + np.bincount(e_dst[~nonself], minlength=n)).astype(np.float32)

    # --- balance graphs across cores by (non-self) edge count (LPT) ---------
    gcnt_e = np.bincount(batch[dst], minlength=G)
    binw = np.zeros(NCORES)
    bins = [[] for _ in range(NCORES)]
    for g in np.argsort(-gcnt_e, kind="stable"):
        c = min((c for c in range(NCORES) if len(bins[c]) < GPC),
                key=lambda c: binw[c])
        bins[c].append(int(g))
        binw[c] += gcnt_e[g]
    graphs_of_core = [sorted(b) for b in bins]
    graph_order = np.array([g for b in graphs_of_core for g in b])

    glo = np.searchsorted(batch, np.arange(G), side="left")
    ghi = np.searchsorted(batch, np.arange(G), side="right")
    indeg = np.bincount(dst, minlength=n)

    core_nodes = [np.concatenate([np.arange(glo[g], ghi[g])
                                  for g in graphs_of_core[c]])
                  for c in range(NCORES)]
    shard_sz = np.array([len(v) for v in core_nodes])
    NP = int(np.ceil(shard_sz.max() / 512.0) * 512)
    NB = NP // 128
    assert NB % 4 == 0
    NR = NB // 4
    GROUP_ROWS = 2 * NP
    assert GROUP_ROWS <= 32767

    # --- within-core relabel: greedy-pack nodes into blocks balancing the
    # per-(block, src-group) in-edge counts (the gather cell sizes).
    # Table layout (for split lo/hi AllGathers): half h of every core is
    # gathered into its own table tile; group(src) = 2*(slot>=H1) + core//4,
    # so a src's group depends on which half its own core placed it in ->
    # iterate the balancing twice.
    # ------------------------------------------------------------------------
    # Table split 50/25/25 — slots [0, HA) AllGathered into tab_a (groups
    # 0-1, ready at ~50% of the layer), [HA, HB) -> tab_b (group 2, ~75%),
    # [HB, NP) -> tab_c (group 3, layer boundary; only this small collective
    # is on the boundary critical path). A node's group = region(core, slot)
    # is fixed by the row math, so regions are pre-assigned (by out-degree)
    # before block balancing.
    HA = NP // 2
    HB = 3 * NP // 4
    core_of_graphnode = np.full(n, -1, np.int64)
    for c in range(NCORES):
        core_of_graphnode[core_nodes[c]] = c
    outdeg = np.bincount(src, minlength=n)

    # region of (core, block) by slot range
    blk_reg = np.empty((NCORES, NB), np.int64)
    for c in range(NCORES):
        for b in range(NB):
            sl = b * 128
            if sl < HA:
                blk_reg[c, b] = (c * HA + sl) // GROUP_ROWS
            elif sl < HB:
                blk_reg[c, b] = 2
            else:
                blk_reg[c, b] = 3
    reg_blocks = [[np.nonzero(blk_reg[c] == r)[0] for r in range(4)]
                  for c in range(NCORES)]

    # pre-assign nodes to regions by out-degree (load ~ capacity)
    region_of = np.full(n, -1, np.int64)
    for c in range(NCORES):
        caps = np.array([len(reg_blocks[c][r]) * 128 for r in range(4)],
                        np.float64)
        od = core_nodes[c][np.argsort(-outdeg[core_nodes[c]], kind="stable")]
        loads = np.zeros(4)
        cnts = np.zeros(4, np.int64)
        for v in od:
            ratio = np.where(cnts < caps, loads / caps, np.inf)
            r = int(np.argmin(ratio))
            region_of[v] = r
            loads[r] += outdeg[v]
            cnts[r] += 1

    egrp_e = region_of[src]                   # [Enon] exact group per edge
    dvec = np.zeros((n, NGROUP), np.int64)
    np.add.at(dvec, (dst, egrp_e), 1)

    core_of_node = np.full(n, -1, np.int64)
    local_of = np.full(n, -1, np.int64)
    node_at = np.full((NCORES, NP), -1, np.int64)   # slot -> old node id
    for c in range(NCORES):
        for r in range(4):
            nodes = core_nodes[c][region_of[core_nodes[c]] == r]
            rb = reg_blocks[c][r]
            nbr = len(rb)
            od = nodes[np.argsort(-indeg[nodes], kind="stable")]
            dv_od = dvec[od].astype(np.float64)       # [nr, NGROUP]
            loads = np.zeros((nbr, NGROUP))
            cnt_b = np.zeros(nbr, np.int64)
            blk = np.empty(len(od), np.int64)
            for i in range(len(od)):
                score = (loads + dv_od[i]).max(axis=1)
                score[cnt_b >= 128] = np.inf
                bi = int(np.argmin(score))
                blk[i] = bi
                loads[bi] += dv_od[i]
                cnt_b[bi] += 1
            cnt_b[:] = 0
            for i in range(len(od)):
                bi = blk[i]
                node_at[c, rb[bi] * 128 + cnt_b[bi]] = od[i]
                cnt_b[bi] += 1
        filled = node_at[c] >= 0
        core_of_node[node_at[c, filled]] = c
        local_of[node_at[c, filled]] = np.nonzero(filled)[0]

    ecore = core_of_node[dst]
    eldst = local_of[dst]                     # local dst slot within shard
    eblk = eldst // 128
    edlocal = (eldst % 128).astype(np.int32)
    s_c = core_of_node[src]
    s_sl = local_of[src]
    QH = NP // 4
    row = np.where(s_sl < HA, s_c * HA + s_sl,
                   np.where(s_sl < HB, s_c * QH + (s_sl - HA),
                            s_c * QH + (s_sl - HB)))
    egrp = np.where(s_sl < HA, row // GROUP_ROWS,
                    np.where(s_sl < HB, 2, 3)).astype(np.int64)
    eidx16 = (row % GROUP_ROWS).astype(np.int32)

    # sort edges by (core, block, group)
    key = ((ecore * NB + eblk) * NGROUP + egrp)
    order = np.argsort(key, kind="stable")
    key_s = key[order]
    idx16_s = eidx16[order]
    dlocal_s = edlocal[order]

    # counts per (core, block, group)
    ncell = NCORES * NB * NGROUP
    cnt = np.bincount(key_s, minlength=ncell).reshape(NCORES, NB, NGROUP)
    # uniform chunk structure: elementwise max across cores
    C_bg = np.ceil(cnt.max(axis=0) / 128.0).astype(np.int64)  # [NB, NGROUP]
    C_b = C_bg.sum(axis=1)                                     # [NB]
    if (C_b == 0).any():
        C_bg[C_b == 0, 0] = 1   # all-pad chunk so the psum accumulator is written
        C_b = C_bg.sum(axis=1)
    TOTCH = int(C_b.sum())

    # segment starts in the sorted edge array
    starts = np.zeros(ncell + 1, np.int64)
    np.cumsum(cnt.reshape(-1), out=starts[1:])

    # --- build per-core padded idx / dst streams --------------------------
    # dst stream (block-major): for b: for g: C_bg[b,g] chunks of 128
    # idx stream  (round/group-major): for r: for g: for b in r: chunks
    dst_cols_off = np.zeros((NB, NGROUP), np.int64)   # col offset of (b,g) in dst stream
    col = 0
    for b in range(NB):
        for g in range(NGROUP):
            dst_cols_off[b, g] = col
            col += C_bg[b, g]
    assert col == TOTCH

    # idx stream offsets per (r, g): columns of 128-idx chunks
    idx_seg_off = np.zeros((NR, NGROUP), np.int64)    # chunk offset of (r,g) seg
    idx_seg_len = np.zeros((NR, NGROUP), np.int64)
    vslot_off = np.zeros((NB, NGROUP), np.int64)      # slot of (b,g) within its (r,g) seg
    ch = 0
    for r in range(NR):
        for g in range(NGROUP):
            idx_seg_off[r, g] = ch
            s = 0
            for b in range(4 * r, 4 * r + 4):
                vslot_off[b, g] = s
                s += C_bg[b, g]
            idx_seg_len[r, g] = s
            ch += s
    assert ch == TOTCH
    CSEG_MAX = int(idx_seg_len.max())

    per_core = []
    for c in range(NCORES):
        idx_stream = np.zeros((TOTCH, 128), np.int16)
        dst_stream = np.full((TOTCH, 128), -1.0, np.float32)
        for b in range(NB):
            for g in range(NGROUP):
                cell = (c * NB + b) * NGROUP + g
                e0, e1 = starts[cell], starts[cell + 1]
                nreal = e1 - e0
                npad = C_bg[b, g] * 128
                vals = np.zeros(npad, np.int16)
                dl = np.full(npad, -1.0, np.float32)
                if nreal > 0:
                    vals[:nreal] = idx16_s[e0:e1].astype(np.int16)
                    vals[nreal:] = vals[nreal - 1]   # duplicate last (row-hit)
                    dl[:nreal] = dlocal_s[e0:e1]
                # chunk layout: chunk j, edge slot p -> stream[row, p]
                vals = vals.reshape(-1, 128)
                dl = dl.reshape(-1, 128)
                # dst stream at block-major cols
                d0 = dst_cols_off[b, g]
                dst_stream[d0:d0 + C_bg[b, g]] = dl
                # idx stream at (r,g)-major cols
                r = b // 4
                i0 = idx_seg_off[r, g] + vslot_off[b, g]
                idx_stream[i0:i0 + C_bg[b, g]] = vals
        # idx DRAM layout for dma_gather: index i of a call at [i%16, i//16].
        # call = contiguous chunk range; within chunk j, slot p: i = j*128+p
        # -> partition (j*128+p)%16 = p%16, column (j*128+p)//16 = j*8 + p//16
        idx_dram = np.zeros((128, TOTCH * 8), np.int16)
        flat = idx_stream.reshape(-1)             # [TOTCH*128]
        ii = np.arange(TOTCH * 128)
        part = (ii % 16).astype(np.int64)
        colx = (ii // 16).astype(np.int64)
        for rep in range(8):
            idx_dram[rep * 16 + part, colx] = flat
        # dst DRAM layout: [128, TOTCH] fp16, partition = edge slot
        dst_dram = dst_stream.T.astype(np.float16).copy()

        # per-node tables (slot-mapped: node_at[c, slot] -> old node id)
        slots = node_at[c]
        fill = slots >= 0
        dv = np.zeros(NP, np.float32)
        dv[fill] = dinv[slots[fill]]
        dinvp = (dv * (1.0 - ALPHA)).reshape(NB, 128).T.copy()   # [128, NB]
        dinvr = dv.reshape(NB, 128).T.copy()                     # [128, NB]
        sc = np.zeros(NP, np.float32)
        sc[fill] = selfcnt[slots[fill]]
        dselfp = (sc * dv * (1.0 - ALPHA)).reshape(NB, 128).T.copy()  # [128, NB]

        # pooling weights: [128, NB, GPC] fp16, value 1/cnt_graph
        garr = np.array(graphs_of_core[c])
        glf = np.searchsorted(garr, batch[slots[fill]])
        gcnt = np.maximum((ghi - glo)[garr].astype(np.float32), 1.0)
        wp = np.zeros((NP, GPC), np.float32)
        wp[np.nonzero(fill)[0], glf] = 1.0 / gcnt[glf]
        wpool = wp.reshape(NB, 128, GPC).transpose(1, 0, 2).copy().astype(np.float16)

        # x^T tiles [KT, 128, NP] fp16 (feat-padded to KT*128)
        KT = (d_in + 127) // 128
        xT = np.zeros((KT * 128, NP), np.float16)
        xT[:d_in, fill] = x[slots[fill]].T.astype(np.float16)
        xT = xT.reshape(KT, 128, NP)

        per_core.append(dict(idx=idx_dram, dstl=dst_dram, dinvp=dinvp,
                             dinvr=dinvr, dselfp=dselfp, wpool=wpool, xT=xT))

    # --- shared weights ---------------------------------------------------
    KT = (d_in + 127) // 128
    linw = np.zeros((KT * 128, D_H), np.float16)
    linw[:d_in] = np.asarray(lin_in_w, np.float32).astype(np.float16)
    linw = linw.reshape(KT, 128, D_H)
    linb = np.asarray(lin_in_b, np.float32)

    betas = np.log(THETA / np.arange(1.0, L_layers + 1.0) + 1.0).astype(np.float32)
    I = np.eye(D_H, dtype=np.float32)
    w1p = np.zeros((L_layers, 2, 128, D_H), np.float16)
    w2p = np.zeros((L_layers, 2, 128, D_H), np.float16)
    for l in range(L_layers):
        b_ = betas[l]
        m1 = b_ * np.asarray(w1[l], np.float32) + (1 - b_) * I
        m2 = ALPHA * (b_ * np.asarray(w2[l], np.float32) + (1 - b_) * I)
        w1p[l] = m1.reshape(2, 128, D_H).astype(np.float16)
        w2p[l] = m2.reshape(2, 128, D_H).astype(np.float16)

    shared = dict(
        linw=linw, linb=linb.reshape(1, D_H),
        w1p=w1p, w2p=w2p,
        lng=np.asarray(ln_g, np.float32)[:L_layers],
        lnb=np.asarray(ln_b, np.float32)[:L_layers],
        c1w=np.asarray(c1_w, np.float32).reshape(2, 128, D_H),
        c1b=np.asarray(c1_b, np.float32).reshape(1, D_H),
        c2w=np.asarray(c2_w, np.float32).reshape(2, 128, D_H // 2),
        c2b=np.asarray(c2_b, np.float32).reshape(1, D_H // 2),
        c3w=np.asarray(c3_w, np.float32).reshape(128, 1),
        c3b=np.asarray(c3_b, np.float32).reshape(1, 1),
    )

    meta = dict(NP=NP, NB=NB, NR=NR, KT=KT, L=L_layers, TOTCH=TOTCH,
                GROUP_ROWS=GROUP_ROWS, CSEG_MAX=CSEG_MAX,
                C_bg=C_bg, C_b=C_b, dst_cols_off=dst_cols_off,
                idx_seg_off=idx_seg_off, idx_seg_len=idx_seg_len,
                vslot_off=vslot_off, graph_order=graph_order)

    in_maps = []
    for c in range(NCORES):
        m = dict(per_core[c])
        m.update(shared)
        in_maps.append(m)
    return in_maps, meta


# ---------------------------------------------------------------------------
# Bass kernel builder
# ---------------------------------------------------------------------------

def build_kernel(meta):
    dbg = set(os.environ.get("KDBG", "").split(",")) - {""}
    NP, NB, NR, KT = meta["NP"], meta["NB"], meta["NR"], meta["KT"]
    Ll, TOTCH = meta["L"], meta["TOTCH"]
    GROUP_ROWS, CSEG_MAX = meta["GROUP_ROWS"], meta["CSEG_MAX"]
    C_bg, C_b = meta["C_bg"], meta["C_b"]
    dst_cols_off = meta["dst_cols_off"]
    idx_seg_off, idx_seg_len = meta["idx_seg_off"], meta["idx_seg_len"]
    vslot_off = meta["vslot_off"]
    CB_MAX = int(C_b.max())

    nc = bacc.Bacc("TRN2", target_bir_lowering=False, debug=False,
                   num_devices=NCORES, num_swdge_queues=4)

    # external inputs
    t_idx = nc.dram_tensor("idx", [128, TOTCH * 8], i16, kind="ExternalInput")
    t_dst = nc.dram_tensor("dstl", [128, TOTCH], f16, kind="ExternalInput")
    t_dinvp = nc.dram_tensor("dinvp", [128, NB], f32, kind="ExternalInput")
    t_dinvr = nc.dram_tensor("dinvr", [128, NB], f32, kind="ExternalInput")
    t_dselfp = nc.dram_tensor("dselfp", [128, NB], f32, kind="ExternalInput")
    t_wpool = nc.dram_tensor("wpool", [128, NB, GPC], f16, kind="ExternalInput")
    t_xT = nc.dram_tensor("xT", [KT, 128, NP], f16, kind="ExternalInput")
    t_linw = nc.dram_tensor("linw", [KT, 128, D_H], f16, kind="ExternalInput")
    t_linb = nc.dram_tensor("linb", [1, D_H], f32, kind="ExternalInput")
    t_w1p = nc.dram_tensor("w1p", [Ll, 2, 128, D_H], f16, kind="ExternalInput")
    t_w2p = nc.dram_tensor("w2p", [Ll, 2, 128, D_H], f16, kind="ExternalInput")
    t_lng = nc.dram_tensor("lng", [Ll, D_H], f32, kind="ExternalInput")
    t_lnb = nc.dram_tensor("lnb", [Ll, D_H], f32, kind="ExternalInput")
    t_c1w = nc.dram_tensor("c1w", [2, 128, D_H], f32, kind="ExternalInput")
    t_c1b = nc.dram_tensor("c1b", [1, D_H], f32, kind="ExternalInput")
    t_c2w = nc.dram_tensor("c2w", [2, 128, D_H // 2], f32, kind="ExternalInput")
    t_c2b = nc.dram_tensor("c2b", [1, D_H // 2], f32, kind="ExternalInput")
    t_c3w = nc.dram_tensor("c3w", [128, 1], f32, kind="ExternalInput")
    t_c3b = nc.dram_tensor("c3b", [1, 1], f32, kind="ExternalInput")
    t_y = nc.dram_tensor("y", [GPC, 1], f32, kind="ExternalOutput")

    def bcast_row(ap_2d, parts=128):
        # [1, D] dram AP -> [parts, D] with 0 partition stride
        return bass.AP(tensor=ap_2d.tensor, offset=ap_2d.offset,
                       ap=[[0, parts]] + list(ap_2d.ap[1:]))

    with tile.TileContext(nc) as tc, ExitStack() as ctx:
        const = ctx.enter_context(tc.tile_pool(name="const", bufs=1))
        resident = ctx.enter_context(tc.tile_pool(name="res", bufs=1))
        work = ctx.enter_context(tc.tile_pool(name="work", bufs=2))
        vpool = ctx.enter_context(tc.tile_pool(name="vpool", bufs=3))
        ohpool = ctx.enter_context(tc.tile_pool(name="ohpool", bufs=2))
        ipool = ctx.enter_context(tc.tile_pool(name="ipool", bufs=2))
        psA = ctx.enter_context(tc.tile_pool(name="psA", bufs=3, space="PSUM"))
        psO = ctx.enter_context(tc.tile_pool(name="psO", bufs=2, space="PSUM"))
        psT = ctx.enter_context(tc.tile_pool(name="psT", bufs=1, space="PSUM"))
        psP = ctx.enter_context(tc.tile_pool(name="psP", bufs=1, space="PSUM"))
        dram = ctx.enter_context(tc.tile_pool(name="dram", bufs=1, space="DRAM"))

        # ---- constants / resident tiles ----------------------------------
        ident32 = const.tile([128, 128], f32)
        make_identity(nc, ident32)
        iota_t = const.tile([128, 128], f16)
        nc.gpsimd.iota(iota_t[:], pattern=[[1, 128]], base=0,
                       channel_multiplier=0,
                       allow_small_or_imprecise_dtypes=True)

        dst_res = resident.tile([128, TOTCH], f16)
        nc.sync.dma_start(out=dst_res, in_=t_dst[:, :])
        dinvp_res = resident.tile([128, NB], f32)
        nc.sync.dma_start(out=dinvp_res, in_=t_dinvp[:, :])
        dselfp_res = resident.tile([128, NB], f32)
        nc.sync.dma_start(out=dselfp_res, in_=t_dselfp[:, :])
        dinvr_res = resident.tile([128, NB], f32)
        nc.sync.dma_start(out=dinvr_res, in_=t_dinvr[:, :])
        wpool_res = resident.tile([128, NB, GPC], f16)
        nc.sync.dma_start(out=wpool_res, in_=t_wpool[:, :, :])
        w1p_res = resident.tile([128, Ll, 2, D_H], f16)
        nc.sync.dma_start(out=w1p_res,
                          in_=t_w1p.rearrange("l k p d -> p l k d"))
        w2p_res = resident.tile([128, Ll, 2, D_H], f16)
        nc.sync.dma_start(out=w2p_res,
                          in_=t_w2p.rearrange("l k p d -> p l k d"))
        linw_res = resident.tile([128, KT, D_H], f16)
        nc.sync.dma_start(out=linw_res, in_=t_linw.rearrange("k p d -> p k d"))
        linb_res = resident.tile([128, D_H], f32)
        nc.gpsimd.dma_start(out=linb_res, in_=bcast_row(t_linb[:, :]))
        lng_res = resident.tile([128, Ll, D_H], f16)
        lnb_res = resident.tile([128, Ll, D_H], f16)
        for l in range(Ll):
            nc.gpsimd.dma_start(out=lng_res[:, l, :], in_=bcast_row(t_lng[l:l + 1, :]))
            nc.gpsimd.dma_start(out=lnb_res[:, l, :], in_=bcast_row(t_lnb[l:l + 1, :]))
        eps_t = const.tile([128, 1], f32)
        nc.vector.memset(eps_t, LN_EPS)

        # DRAM intermediates (fp8 gather tables, split 50/25/25 so only the
        # small last AllGather remains on the layer-boundary critical path)
        HA = NP // 2
        HB = 3 * NP // 4
        QH = NP // 4
        ag_in = [dram.tile([NP, D_H], f8, name=f"ag_in_{l}") for l in range(Ll)]
        tab_a = [dram.tile([NCORES * HA, D_H], f8, addr_space="Shared",
                           name=f"taba_{l}") for l in range(Ll)]
        tab_b = [dram.tile([NCORES * QH, D_H], f8, addr_space="Shared",
                           name=f"tabb_{l}") for l in range(Ll)]
        tab_c = [dram.tile([NCORES * QH, D_H], f8, addr_space="Shared",
                           name=f"tabc_{l}") for l in range(Ll)]

        # SBUF residents: h0^T (lhsT for the W2'' term) and hs = dinv*h of the
        # current layer (diagonal/self-loop term), avoiding DRAM round-trips
        h0T_res = resident.tile([128, NB, 2, 128], f16)
        hs_res = resident.tile([128, NB, D_H], f8)

        # ---- input layer: h0 = relu(x @ linw + b), write hs0 = dinv*h0 ---
        for b in range(NB):
            xt = work.tile([128, KT, 128], f16, tag="xstage")
            nc.sync.dma_start(out=xt, in_=t_xT[:, :, b * 128:(b + 1) * 128]
                              .rearrange("k p n -> p k n"))
            ps = psO.tile([128, D_H], f32, tag="outp")
            for k in range(KT):
                nc.tensor.matmul(ps[:], lhsT=xt[:, k, :], rhs=linw_res[:, k, :],
                                 start=(k == 0), stop=(k == KT - 1))
            # relu(ps + bias) : add bias on DVE, relu on ACT (fp32 for transpose)
            tmp = work.tile([128, D_H], f32, tag="lntmp")
            nc.vector.tensor_add(out=tmp[:], in0=ps[:], in1=linb_res[:])
            h0 = work.tile([128, D_H], f32, tag="h0f")
            nc.scalar.activation(out=h0[:], in_=tmp[:],
                                 func=mybir.ActivationFunctionType.Relu)
            # h0T -> resident SBUF (lhsT for the W2'' term every layer)
            trp = psT.tile([128, 256], f32, tag="trp")
            nc.tensor.transpose(out=trp[:, 0:128], in_=h0[:, 0:128], identity=ident32[:])
            nc.tensor.transpose(out=trp[:, 128:256], in_=h0[:, 128:256], identity=ident32[:])
            nc.vector.tensor_copy(out=h0T_res[:, b, 0, :], in_=trp[:, 0:128])
            nc.vector.tensor_copy(out=h0T_res[:, b, 1, :], in_=trp[:, 128:256])
            # hs0 = dinv * h0 -> resident + ag_in[0]
            nc.scalar.activation(out=hs_res[:, b, :], in_=h0[:],
                                 func=mybir.ActivationFunctionType.Identity,
                                 scale=dinvr_res[:, b:b + 1])
            nc.sync.dma_start(out=ag_in[0][b * 128:(b + 1) * 128, :],
                              in_=hs_res[:, b, :])

        pool_ps = psP.tile([GPC, D_H], f32)
        gq = [0]  # swdge queue rotation counter

        # ---- layers -------------------------------------------------------
        for l in range(Ll):
            if "nocc" not in dbg:
                for lo_, hi_, tabt in ((0, HA, tab_a), (HA, HB, tab_b),
                                       (HB, NP, tab_c)):
                    nc.gpsimd.collective_compute(
                        "AllGather", mybir.AluOpType.bypass,
                        ins=[ag_in[l][lo_:hi_]], outs=[tabt[l][:]],
                        replica_groups=[list(range(NCORES))],
                    )

            for r in range(NR):
                # gather stage for this round: one dma_gather per group
                vt = {}
                for g in range(NGROUP):
                    seg = int(idx_seg_len[r, g])
                    if seg == 0:
                        continue
                    nidx = seg * 128
                    it = ipool.tile([128, CSEG_MAX * 8], i16, tag=f"idx{g}")
                    c0 = int(idx_seg_off[r, g]) * 8
                    nc.sync.dma_start(out=it[:, :seg * 8],
                                      in_=t_idx[:, c0:c0 + seg * 8])
                    v = vpool.tile([128, CSEG_MAX, D_H], f8, tag=f"v{g}")
                    if "nogather" in dbg:
                        nc.vector.memset(v[:, :seg, :], 0.25)
                    else:
                        tab = (tab_a[l] if g < 2 else
                               tab_b[l] if g == 2 else tab_c[l])
                        goff = (g % 2) * GROUP_ROWS if g < 2 else 0
                        for s0 in range(0, seg, 8):
                            sub = min(8, seg - s0)
                            nc.gpsimd.dma_gather(
                                v[:, s0:s0 + sub, :],
                                tab[goff:goff + GROUP_ROWS, :],
                                it[:, s0 * 8:(s0 + sub) * 8],
                                num_idxs=sub * 128,
                                num_idxs_reg=sub * 128,
                                elem_size=D_H,
                                queue_num=gq[0] % 4,
                            )
                            gq[0] += 1
                    vt[g] = v

                for b in range(4 * r, 4 * r + 4):
                    cb = int(C_b[b])
                    # one-hot for the whole block: [128, cb, 128] fp16
                    oh = ohpool.tile([128, CB_MAX, 128], f8, tag="oh")
                    d0 = int(dst_cols_off[b, 0])
                    dst_sl = dst_res[:, d0:d0 + cb]
                    dst_b = bass.AP(tensor=dst_sl.tensor, offset=dst_sl.offset,
                                    ap=[dst_sl.ap[0], dst_sl.ap[1], [0, 128]])
                    io_sl = iota_t[:, :]
                    iota_b = bass.AP(tensor=io_sl.tensor, offset=io_sl.offset,
                                     ap=[io_sl.ap[0], [0, cb], io_sl.ap[1]])
                    if "nooh" in dbg:
                        nc.vector.memset(oh[:, :cb, :], 0.0)
                    else:
                        nc.vector.tensor_tensor(out=oh[:, :cb, :], in0=dst_b,
                                                in1=iota_b, op=mybir.AluOpType.is_equal)

                    # segment-sum into psum
                    aps = psA.tile([128, D_H], f32, tag="acc")
                    if "noseg" in dbg:
                        nc.vector.memset(aps[:], 0.125)
                    else:
                        mm = 0
                        for g in range(NGROUP):
                            cbg = int(C_bg[b, g])
                            for j in range(cbg):
                                ohcol = int(dst_cols_off[b, g]) - d0 + j
                                vslot = int(vslot_off[b, g]) + j
                                nc.tensor.matmul(
                                    aps[:], lhsT=oh[:, ohcol, :],
                                    rhs=vt[g][:, vslot, :],
                                    start=(mm == 0), stop=(mm == cb - 1),
                                )
                                mm += 1
                    # a = (1-alpha)*dinv_dst * psum  (fp32, on ACT)
                    a_sb = work.tile([128, D_H], f32, tag="asb")
                    nc.scalar.activation(out=a_sb[:], in_=aps[:],
                                         func=mybir.ActivationFunctionType.Identity,
                                         scale=dinvp_res[:, b:b + 1])
                    # + diagonal (self-loop) term: dselfp * hs_prev (resident)
                    a_sb2 = work.tile([128, D_H], f32, tag="asb2")
                    nc.vector.scalar_tensor_tensor(
                        out=a_sb2[:], in0=hs_res[:, b, :],
                        scalar=dselfp_res[:, b:b + 1], in1=a_sb[:],
                        op0=mybir.AluOpType.mult, op1=mybir.AluOpType.add)
                    # aT via PE transpose (fp32 psum; fp16 psum reads are slow)
                    trp = psT.tile([128, 256], f32, tag="trp")
                    nc.tensor.transpose(out=trp[:, 0:128], in_=a_sb2[:, 0:128],
                                        identity=ident32[:])
                    nc.tensor.transpose(out=trp[:, 128:256], in_=a_sb2[:, 128:256],
                                        identity=ident32[:])
                    aT = work.tile([128, 2, 128], f16, tag="aT")
                    nc.vector.tensor_copy(out=aT[:, 0, :], in_=trp[:, 0:128])
                    nc.vector.tensor_copy(out=aT[:, 1, :], in_=trp[:, 128:256])

                    # out = a @ W1' + h0 @ W2''
                    ops = psO.tile([128, D_H], f32, tag="outp")
                    nc.tensor.matmul(ops[:], lhsT=aT[:, 0, :],
                                     rhs=w1p_res[:, l, 0, :], start=True, stop=False)
                    nc.tensor.matmul(ops[:], lhsT=aT[:, 1, :],
                                     rhs=w1p_res[:, l, 1, :], start=False, stop=False)
                    nc.tensor.matmul(ops[:], lhsT=h0T_res[:, b, 0, :],
                                     rhs=w2p_res[:, l, 0, :], start=False, stop=False)
                    nc.tensor.matmul(ops[:], lhsT=h0T_res[:, b, 1, :],
                                     rhs=w2p_res[:, l, 1, :], start=False, stop=True)

                    # LayerNorm + gamma/beta + relu
                    stats = work.tile([128, 6], f32, tag="stats")
                    nc.vector.bn_stats(out=stats[:], in_=ops[:])
                    mv = work.tile([128, 2], f32, tag="mv")
                    nc.vector.bn_aggr(out=mv[:], in_=stats[:])
                    rstd = work.tile([128, 1], f32, tag="rstd")
                    nc.scalar.activation(out=rstd[:], in_=mv[:, 1:2],
                                         func=mybir.ActivationFunctionType.Abs_reciprocal_sqrt,
                                         bias=eps_t[:], scale=1.0)
                    nmr = work.tile([128, 1], f32, tag="nmr")
                    nc.vector.tensor_scalar(out=nmr[:], in0=mv[:, 0:1],
                                            scalar1=rstd[:], scalar2=-1.0,
                                            op0=mybir.AluOpType.mult,
                                            op1=mybir.AluOpType.mult)
                    normed = work.tile([128, D_H], f16, tag="normed")
                    nc.scalar.activation(out=normed[:], in_=ops[:],
                                         func=mybir.ActivationFunctionType.Identity,
                                         bias=nmr[:], scale=rstd[:])
                    # gamma * normed + beta, then relu
                    gb = work.tile([128, D_H], f16, tag="gb")
                    nc.vector.scalar_tensor_tensor(
                        out=gb[:], in0=normed[:], scalar=1.0,
                        in1=lng_res[:, l, :],
                        op0=mybir.AluOpType.mult, op1=mybir.AluOpType.mult)
                    hn = work.tile([128, D_H], f16, tag="hn")
                    nc.vector.tensor_tensor(out=hn[:], in0=gb[:],
                                            in1=lnb_res[:, l, :],
                                            op=mybir.AluOpType.add)
                    hr = work.tile([128, D_H], f16, tag="hr")
                    nc.scalar.activation(out=hr[:], in_=hn[:],
                                         func=mybir.ActivationFunctionType.Relu)

                    if l == Ll - 1:
                        if "nopool" not in dbg:
                            nc.tensor.matmul(pool_ps[:], lhsT=wpool_res[:, b, :],
                                             rhs=hr[:], start=(b == 0),
                                             stop=(b == NB - 1),
                                             skip_group_check=True)
                    else:
                        nc.scalar.activation(out=hs_res[:, b, :], in_=hr[:],
                                             func=mybir.ActivationFunctionType.Identity,
                                             scale=dinvr_res[:, b:b + 1])
                        nc.sync.dma_start(
                            out=ag_in[l + 1][b * 128:(b + 1) * 128, :],
                            in_=hs_res[:, b, :])

        # ---- head ---------------------------------------------------------
        pooled = work.tile([GPC, D_H], f32, tag="pooled")
        if "nopool" in dbg:
            nc.vector.memset(pooled[:], 0.5)
        else:
            nc.vector.tensor_copy(out=pooled[:], in_=pool_ps[:])

        def head_mm(z, kdim, wtile_list, btile, relu, outdim):
            # z: [GPC, kdim] fp32 sbuf -> out [GPC, outdim] fp32
            trp2 = psT.tile([128, 256], f32, tag="trp")
            zT = work.tile([128, (kdim + 127) // 128, GPC], f32, tag="zT")
            for k in range((kdim + 127) // 128):
                kk = min(128, kdim - k * 128)
                nc.tensor.transpose(out=trp2[:kk, k * 128:k * 128 + GPC],
                                    in_=z[:, k * 128:k * 128 + kk],
                                    identity=ident32[:GPC, :GPC])
                nc.vector.tensor_copy(out=zT[:kk, k, :],
                                      in_=trp2[:kk, k * 128:k * 128 + GPC])
            ps = psO.tile([GPC, max(outdim, 1)], f32, tag="headp", bufs=1)
            nk = (kdim + 127) // 128
            for k in range(nk):
                kk = min(128, kdim - k * 128)
                nc.tensor.matmul(ps[:], lhsT=zT[:kk, k, :], rhs=wtile_list[k][:kk, :],
                                 start=(k == 0), stop=(k == nk - 1))
            zo = work.tile([GPC, outdim], f32, tag="zo")
            nc.vector.tensor_add(out=zo[:], in0=ps[:], in1=btile[:GPC, :])
            if relu:
                zr = work.tile([GPC, outdim], f32, tag="zr")
                nc.scalar.activation(out=zr[:], in_=zo[:],
                                     func=mybir.ActivationFunctionType.Relu)
                return zr
            return zo

        c1w_t = resident.tile([128, 2, D_H], f32)
        nc.sync.dma_start(out=c1w_t, in_=t_c1w.rearrange("k p d -> p k d"))
        c1b_t = resident.tile([128, D_H], f32)
        nc.gpsimd.dma_start(out=c1b_t, in_=bcast_row(t_c1b[:, :]))
        c2w_t = resident.tile([128, 2, D_H // 2], f32)
        nc.sync.dma_start(out=c2w_t, in_=t_c2w.rearrange("k p d -> p k d"))
        c2b_t = resident.tile([128, D_H // 2], f32)
        nc.gpsimd.dma_start(out=c2b_t, in_=bcast_row(t_c2b[:, :]))
        c3w_t = resident.tile([128, 1], f32)
        nc.sync.dma_start(out=c3w_t, in_=t_c3w[:, :])
        c3b_t = resident.tile([128, 1], f32)
        nc.gpsimd.dma_start(out=c3b_t, in_=bcast_row(t_c3b[:, :]))

        z1 = head_mm(pooled, D_H, [c1w_t[:, 0, :], c1w_t[:, 1, :]], c1b_t, True, D_H)
        z2 = head_mm(z1, D_H, [c2w_t[:, 0, :], c2w_t[:, 1, :]], c2b_t, True, D_H // 2)
        z3 = head_mm(z2, D_H // 2, [c3w_t], c3b_t, False, 1)
        nc.sync.dma_start(out=t_y[:, :], in_=z3[:])

    nc.compile()
    return nc


# ---------------------------------------------------------------------------
# entry point
# ---------------------------------------------------------------------------

_CACHE = {}


def kernel(**inputs):
    global LAST_EXEC_NS
    trace = bool(os.environ.get("BASS_TRACE"))
    if trace:
        _maybe_register_ntff_hook()

    in_maps, meta = preprocess(**inputs)
    ckey = ("k", meta["NP"], meta["TOTCH"])
    if ckey not in _CACHE:
        _CACHE[ckey] = build_kernel(meta)
    nc = _CACHE[ckey]

    res = run_bass_kernel_spmd(nc, in_maps, core_ids=list(range(NCORES)),
                               trace=trace)
    LAST_EXEC_NS = res.exec_time_ns
    ycat = np.concatenate([res.results[c]["y"].reshape(-1)
                           for c in range(NCORES)])
    y = np.empty(G, np.float32)
    y[meta["graph_order"]] = ycat
    return y



# revision 52
# speedup vs baseline: 1.1556x; 1.1556x over previous
"""GCN2 (8-layer, N=100K, E=1.6M, G=128) Trainium2 Bass kernel, 8-core SPMD.

Strategy (data-parallel over graphs, per sharding hint):
- batch is sorted => graphs are block-contiguous in node ids. 16 graphs/core.
- Per layer: each core computes h for its node shard; shards are AllGathered
  into a full fp16 table [8*NP, 256] in Shared DRAM; the edge aggregation
  A_hat @ h is done per-core over edges grouped by destination:
    * edges sorted into (dst-block of 128 nodes) x (src-range group of <=2*NP
      rows, so gather indices fit int16), padded to chunks of 128 edges
    * dma_gather pulls 128 rows (512B fp16 each) per chunk from the table
    * a one-hot matmul (lhsT = onehot[edge_slot, dst_local], rhs = gathered
      rows) segment-sums each chunk into the block's PSUM accumulator
  The symmetric gcn_norm is folded in: table rows are pre-scaled by
  dinv[src]; dinv[dst]*(1-alpha) is applied per-partition when copying the
  PSUM accumulator out.
- GCN2Conv update: out = a @ W1' + h0 @ W2'' with W1' = b*W1 + (1-b)*I,
  W2'' = ALPHA*(b*W2 + (1-b)*I) folded on host. h0^T is kept resident in
  SBUF (fp16) as the stationary operand; a is transposed on the PE.
- LayerNorm via bn_stats/bn_aggr (fp32), then gamma/beta + ReLU.
- Mean-pool folded into a per-block one-hot matmul on the last layer
  (weights 1/cnt), then a tiny per-core MLP head -> y[16] per core.

The instruction stream must be identical on all 8 cores (SPMD): per-(block,
group) chunk counts are the elementwise MAX across cores; shorter cores pad
with duplicate gather indices and dst_local=-1 (one-hot row of zeros).
"""

import os
import sys
import types
import numpy as np
from contextlib import ExitStack

sys.path.insert(0, "/opt/trn_rl_repo")
if os.path.isdir("/root/.axon_site"):
    sys.path.insert(0, "/root/.axon_site")

import concourse.bass as bass
import concourse.bacc as bacc
import concourse.tile as tile
from concourse import mybir
from concourse.bass_utils import run_bass_kernel_spmd
from concourse.masks import make_identity

f16 = mybir.dt.float16
f32 = mybir.dt.float32
f8 = mybir.dt.float8e4
i16 = mybir.dt.int16

# problem constants
N, E, G = 100000, 1600000, 128
D_IN, D_H, L = 771, 256, 8
ALPHA, THETA = 0.1, 0.5
LN_EPS = 1e-5
NCORES = 8
GPC = G // NCORES          # graphs per core
NGROUP = 4                 # src-range groups (int16 gather index limit)

LAST_EXEC_NS = None


def _maybe_register_ntff_hook():
    """Register the axon NTFF profiling hook if the image's antenv lacks it."""
    try:
        from antenv import axon_hooks  # noqa: F401
        return
    except ImportError:
        pass
    try:
        from trn_agent_boot.trn_boot import _ntff_profile_via_ctypes
        import antenv
        mod = types.ModuleType("antenv.axon_hooks")
        hook = _ntff_profile_via_ctypes("/opt/axon/libaxon_pjrt.so")
        if hook is None:
            return
        mod.get_axon_ntff_profile_hook = lambda: hook
        sys.modules["antenv.axon_hooks"] = mod
        antenv.axon_hooks = mod
    except Exception:
        pass


# ---------------------------------------------------------------------------
# Host preprocessing
# ---------------------------------------------------------------------------

def preprocess(x, edge_index, batch, lin_in_w, lin_in_b, w1, w2, ln_g, ln_b,
               c1_w, c1_b, c2_w, c2_b, c3_w, c3_b, L_layers=L):
    x = np.asarray(x, np.float32)
    edge_index = np.asarray(edge_index)
    batch = np.asarray(batch)
    n = x.shape[0]
    d_in = x.shape[1]

    # degrees include the gcn_norm self-loop; self-loop edges (added + any
    # natural src==dst edges) are handled by a local diagonal term on-device,
    # so only src!=dst edges enter the gather streams.
    e_src = edge_index[0].astype(np.int64)
    e_dst = edge_index[1].astype(np.int64)
    deg = (np.bincount(e_dst, minlength=n) + 1).astype(np.float64)
    dinv = (1.0 / np.sqrt(deg)).astype(np.float32)  # deg >= 1 via self-loop
    nonself = e_src != e_dst
    src = e_src[nonself]
    dst = e_dst[nonself]
    selfcnt = (1 + np.bincount(e_dst[~nonself], minlength=n)).astype(np.float32)

    # --- balance graphs across cores by (non-self) edge count (LPT) ---------
    gcnt_e = np.bincount(batch[dst], minlength=G)
    binw = np.zeros(NCORES)
    bins = [[] for _ in range(NCORES)]
    for g in np.argsort(-gcnt_e, kind="stable"):
        c = min((c for c in range(NCORES) if len(bins[c]) < GPC),
                key=lambda c: binw[c])
        bins[c].append(int(g))
        binw[c] += gcnt_e[g]
    graphs_of_core = [sorted(b) for b in bins]
    graph_order = np.array([g for b in graphs_of_core for g in b])

    glo = np.searchsorted(batch, np.arange(G), side="left")
    ghi = np.searchsorted(batch, np.arange(G), side="right")
    indeg = np.bincount(dst, minlength=n)

    core_nodes = [np.concatenate([np.arange(glo[g], ghi[g])
                                  for g in graphs_of_core[c]])
                  for c in range(NCORES)]
    shard_sz = np.array([len(v) for v in core_nodes])
    NP = int(np.ceil(shard_sz.max() / 512.0) * 512)
    NB = NP // 128
    assert NB % 4 == 0
    NR = NB // 4
    GROUP_ROWS = 2 * NP
    assert GROUP_ROWS <= 32767

    # --- within-core relabel: greedy-pack nodes into blocks balancing the
    # per-(block, src-group) in-edge counts (the gather cell sizes).
    # Table layout (for split lo/hi AllGathers): half h of every core is
    # gathered into its own table tile; group(src) = 2*(slot>=H1) + core//4,
    # so a src's group depends on which half its own core placed it in ->
    # iterate the balancing twice.
    # ------------------------------------------------------------------------
    # Table split in slot halves: lo half of every core's shard AllGathered
    # into tab_lo (groups 0-1, ready at ~50% of the layer so the collective
    # hides under block compute), hi half -> tab_hi (groups 2-3). A node's
    # group = region(core, slot) is fixed by the row math, so regions are
    # pre-assigned (by out-degree) before block balancing.
    HA = NP // 2
    core_of_graphnode = np.full(n, -1, np.int64)
    for c in range(NCORES):
        core_of_graphnode[core_nodes[c]] = c
    outdeg = np.bincount(src, minlength=n)

    # region of (core, block) by slot range: lo -> 2*half + core//4 pattern
    blk_reg = np.empty((NCORES, NB), np.int64)
    for c in range(NCORES):
        for b in range(NB):
            sl = b * 128
            h = 0 if sl < HA else 1
            blk_reg[c, b] = 2 * h + c // 4
    reg_blocks = [[np.nonzero(blk_reg[c] == r)[0] for r in range(4)]
                  for c in range(NCORES)]

    # pre-assign nodes to regions by out-degree (load ~ capacity)
    region_of = np.full(n, -1, np.int64)
    for c in range(NCORES):
        caps = np.array([len(reg_blocks[c][r]) * 128 for r in range(4)],
                        np.float64)
        od = core_nodes[c][np.argsort(-outdeg[core_nodes[c]], kind="stable")]
        loads = np.zeros(4)
        cnts = np.zeros(4, np.int64)
        for v in od:
            ratio = np.where(cnts < caps, loads / caps, np.inf)
            r = int(np.argmin(ratio))
            region_of[v] = r
            loads[r] += outdeg[v]
            cnts[r] += 1

    egrp_e = region_of[src]                   # [Enon] exact group per edge
    dvec = np.zeros((n, NGROUP), np.int64)
    np.add.at(dvec, (dst, egrp_e), 1)

    core_of_node = np.full(n, -1, np.int64)
    local_of = np.full(n, -1, np.int64)
    node_at = np.full((NCORES, NP), -1, np.int64)   # slot -> old node id
    for c in range(NCORES):
        for r in range(4):
            nodes = core_nodes[c][region_of[core_nodes[c]] == r]
            rb = reg_blocks[c][r]
            nbr = len(rb)
            od = nodes[np.argsort(-indeg[nodes], kind="stable")]
            dv_od = dvec[od].astype(np.float64)       # [nr, NGROUP]
            loads = np.zeros((nbr, NGROUP))
            cnt_b = np.zeros(nbr, np.int64)
            blk = np.empty(len(od), np.int64)
            for i in range(len(od)):
                score = (loads + dv_od[i]).max(axis=1)
                score[cnt_b >= 128] = np.inf
                bi = int(np.argmin(score))
                blk[i] = bi
                loads[bi] += dv_od[i]
                cnt_b[bi] += 1
            cnt_b[:] = 0
            for i in range(len(od)):
                bi = blk[i]
                node_at[c, rb[bi] * 128 + cnt_b[bi]] = od[i]
                cnt_b[bi] += 1
        filled = node_at[c] >= 0
        core_of_node[node_at[c, filled]] = c
        local_of[node_at[c, filled]] = np.nonzero(filled)[0]

    ecore = core_of_node[dst]
    eldst = local_of[dst]                     # local dst slot within shard
    eblk = eldst // 128
    edlocal = (eldst % 128).astype(np.int32)
    s_c = core_of_node[src]
    s_sl = local_of[src]
    s_half = s_sl // HA
    egrp = (2 * s_half + s_c // 4).astype(np.int64)
    eidx16 = ((s_c % 4) * HA + s_sl % HA).astype(np.int32)

    # sort edges by (core, block, group)
    key = ((ecore * NB + eblk) * NGROUP + egrp)
    order = np.argsort(key, kind="stable")
    key_s = key[order]
    idx16_s = eidx16[order]
    dlocal_s = edlocal[order]

    # counts per (core, block, group)
    ncell = NCORES * NB * NGROUP
    cnt = np.bincount(key_s, minlength=ncell).reshape(NCORES, NB, NGROUP)
    # uniform chunk structure: elementwise max across cores
    C_bg = np.ceil(cnt.max(axis=0) / 128.0).astype(np.int64)  # [NB, NGROUP]
    C_b = C_bg.sum(axis=1)                                     # [NB]
    if (C_b == 0).any():
        C_bg[C_b == 0, 0] = 1   # all-pad chunk so the psum accumulator is written
        C_b = C_bg.sum(axis=1)
    TOTCH = int(C_b.sum())

    # segment starts in the sorted edge array
    starts = np.zeros(ncell + 1, np.int64)
    np.cumsum(cnt.reshape(-1), out=starts[1:])

    # --- build per-core padded idx / dst streams --------------------------
    # dst stream (block-major): for b: for g: C_bg[b,g] chunks of 128
    # idx stream  (round/group-major): for r: for g: for b in r: chunks
    dst_cols_off = np.zeros((NB, NGROUP), np.int64)   # col offset of (b,g) in dst stream
    col = 0
    for b in range(NB):
        for g in range(NGROUP):
            dst_cols_off[b, g] = col
            col += C_bg[b, g]
    assert col == TOTCH

    # idx stream offsets per (r, g): columns of 128-idx chunks
    idx_seg_off = np.zeros((NR, NGROUP), np.int64)    # chunk offset of (r,g) seg
    idx_seg_len = np.zeros((NR, NGROUP), np.int64)
    vslot_off = np.zeros((NB, NGROUP), np.int64)      # slot of (b,g) within its (r,g) seg
    ch = 0
    for r in range(NR):
        for g in range(NGROUP):
            idx_seg_off[r, g] = ch
            s = 0
            for b in range(4 * r, 4 * r + 4):
                vslot_off[b, g] = s
                s += C_bg[b, g]
            idx_seg_len[r, g] = s
            ch += s
    assert ch == TOTCH
    CSEG_MAX = int(idx_seg_len.max())

    per_core = []
    for c in range(NCORES):
        idx_stream = np.zeros((TOTCH, 128), np.int16)
        dst_stream = np.full((TOTCH, 128), -1.0, np.float32)
        for b in range(NB):
            for g in range(NGROUP):
                cell = (c * NB + b) * NGROUP + g
                e0, e1 = starts[cell], starts[cell + 1]
                nreal = e1 - e0
                npad = C_bg[b, g] * 128
                vals = np.zeros(npad, np.int16)
                dl = np.full(npad, -1.0, np.float32)
                if nreal > 0:
                    vals[:nreal] = idx16_s[e0:e1].astype(np.int16)
                    vals[nreal:] = vals[nreal - 1]   # duplicate last (row-hit)
                    dl[:nreal] = dlocal_s[e0:e1]
                # chunk layout: chunk j, edge slot p -> stream[row, p]
                vals = vals.reshape(-1, 128)
                dl = dl.reshape(-1, 128)
                # dst stream at block-major cols
                d0 = dst_cols_off[b, g]
                dst_stream[d0:d0 + C_bg[b, g]] = dl
                # idx stream at (r,g)-major cols
                r = b // 4
                i0 = idx_seg_off[r, g] + vslot_off[b, g]
                idx_stream[i0:i0 + C_bg[b, g]] = vals
        # idx DRAM layout for dma_gather: index i of a call at [i%16, i//16].
        # call = contiguous chunk range; within chunk j, slot p: i = j*128+p
        # -> partition (j*128+p)%16 = p%16, column (j*128+p)//16 = j*8 + p//16
        idx_dram = np.zeros((128, TOTCH * 8), np.int16)
        flat = idx_stream.reshape(-1)             # [TOTCH*128]
        ii = np.arange(TOTCH * 128)
        part = (ii % 16).astype(np.int64)
        colx = (ii // 16).astype(np.int64)
        for rep in range(8):
            idx_dram[rep * 16 + part, colx] = flat
        # dst DRAM layout: [128, TOTCH] fp16, partition = edge slot
        dst_dram = dst_stream.T.astype(np.float16).copy()

        # per-node tables (slot-mapped: node_at[c, slot] -> old node id)
        slots = node_at[c]
        fill = slots >= 0
        dv = np.zeros(NP, np.float32)
        dv[fill] = dinv[slots[fill]]
        dinvp = (dv * (1.0 - ALPHA)).reshape(NB, 128).T.copy()   # [128, NB]
        dinvr = dv.reshape(NB, 128).T.copy()                     # [128, NB]
        sc = np.zeros(NP, np.float32)
        sc[fill] = selfcnt[slots[fill]]
        dselfp = (sc * dv * (1.0 - ALPHA)).reshape(NB, 128).T.copy()  # [128, NB]

        # pooling weights: [128, NB, GPC] fp16, value 1/cnt_graph
        garr = np.array(graphs_of_core[c])
        glf = np.searchsorted(garr, batch[slots[fill]])
        gcnt = np.maximum((ghi - glo)[garr].astype(np.float32), 1.0)
        wp = np.zeros((NP, GPC), np.float32)
        wp[np.nonzero(fill)[0], glf] = 1.0 / gcnt[glf]
        wpool = wp.reshape(NB, 128, GPC).transpose(1, 0, 2).copy().astype(np.float16)

        # x^T tiles [KT, 128, NP] fp16 (feat-padded to KT*128)
        KT = (d_in + 127) // 128
        xT = np.zeros((KT * 128, NP), np.float16)
        xT[:d_in, fill] = x[slots[fill]].T.astype(np.float16)
        xT = xT.reshape(KT, 128, NP)

        per_core.append(dict(idx=idx_dram, dstl=dst_dram, dinvp=dinvp,
                             dinvr=dinvr, dselfp=dselfp, wpool=wpool, xT=xT))

    # --- shared weights ---------------------------------------------------
    KT = (d_in + 127) // 128
    linw = np.zeros((KT * 128, D_H), np.float16)
    linw[:d_in] = np.asarray(lin_in_w, np.float32).astype(np.float16)
    linw = linw.reshape(KT, 128, D_H)
    linb = np.asarray(lin_in_b, np.float32)

    betas = np.log(THETA / np.arange(1.0, L_layers + 1.0) + 1.0).astype(np.float32)
    I = np.eye(D_H, dtype=np.float32)
    w1p = np.zeros((L_layers, 2, 128, D_H), np.float16)
    w2p = np.zeros((L_layers, 2, 128, D_H), np.float16)
    for l in range(L_layers):
        b_ = betas[l]
        m1 = b_ * np.asarray(w1[l], np.float32) + (1 - b_) * I
        m2 = ALPHA * (b_ * np.asarray(w2[l], np.float32) + (1 - b_) * I)
        w1p[l] = m1.reshape(2, 128, D_H).astype(np.float16)
        w2p[l] = m2.reshape(2, 128, D_H).astype(np.float16)

    shared = dict(
        linw=linw, linb=linb.reshape(1, D_H),
        w1p=w1p, w2p=w2p,
        lng=np.asarray(ln_g, np.float32)[:L_layers],
        lnb=np.asarray(ln_b, np.float32)[:L_layers],
        c1w=np.asarray(c1_w, np.float32).reshape(2, 128, D_H),
        c1b=np.asarray(c1_b, np.float32).reshape(1, D_H),
        c2w=np.asarray(c2_w, np.float32).reshape(2, 128, D_H // 2),
        c2b=np.asarray(c2_b, np.float32).reshape(1, D_H // 2),
        c3w=np.asarray(c3_w, np.float32).reshape(128, 1),
        c3b=np.asarray(c3_b, np.float32).reshape(1, 1),
    )

    meta = dict(NP=NP, NB=NB, NR=NR, KT=KT, L=L_layers, TOTCH=TOTCH,
                GROUP_ROWS=GROUP_ROWS, CSEG_MAX=CSEG_MAX,
                C_bg=C_bg, C_b=C_b, dst_cols_off=dst_cols_off,
                idx_seg_off=idx_seg_off, idx_seg_len=idx_seg_len,
                vslot_off=vslot_off, graph_order=graph_order)

    in_maps = []
    for c in range(NCORES):
        m = dict(per_core[c])
        m.update(shared)
        in_maps.append(m)
    return in_maps, meta


# ---------------------------------------------------------------------------
# Bass kernel builder
# ---------------------------------------------------------------------------

def build_kernel(meta):
    dbg = set(os.environ.get("KDBG", "").split(",")) - {""}
    NP, NB, NR, KT = meta["NP"], meta["NB"], meta["NR"], meta["KT"]
    Ll, TOTCH = meta["L"], meta["TOTCH"]
    GROUP_ROWS, CSEG_MAX = meta["GROUP_ROWS"], meta["CSEG_MAX"]
    C_bg, C_b = meta["C_bg"], meta["C_b"]
    dst_cols_off = meta["dst_cols_off"]
    idx_seg_off, idx_seg_len = meta["idx_seg_off"], meta["idx_seg_len"]
    vslot_off = meta["vslot_off"]
    CB_MAX = int(C_b.max())

    nc = bacc.Bacc("TRN2", target_bir_lowering=False, debug=False,
                   num_devices=NCORES, num_swdge_queues=4)

    # external inputs
    t_idx = nc.dram_tensor("idx", [128, TOTCH * 8], i16, kind="ExternalInput")
    t_dst = nc.dram_tensor("dstl", [128, TOTCH], f16, kind="ExternalInput")
    t_dinvp = nc.dram_tensor("dinvp", [128, NB], f32, kind="ExternalInput")
    t_dinvr = nc.dram_tensor("dinvr", [128, NB], f32, kind="ExternalInput")
    t_dselfp = nc.dram_tensor("dselfp", [128, NB], f32, kind="ExternalInput")
    t_wpool = nc.dram_tensor("wpool", [128, NB, GPC], f16, kind="ExternalInput")
    t_xT = nc.dram_tensor("xT", [KT, 128, NP], f16, kind="ExternalInput")
    t_linw = nc.dram_tensor("linw", [KT, 128, D_H], f16, kind="ExternalInput")
    t_linb = nc.dram_tensor("linb", [1, D_H], f32, kind="ExternalInput")
    t_w1p = nc.dram_tensor("w1p", [Ll, 2, 128, D_H], f16, kind="ExternalInput")
    t_w2p = nc.dram_tensor("w2p", [Ll, 2, 128, D_H], f16, kind="ExternalInput")
    t_lng = nc.dram_tensor("lng", [Ll, D_H], f32, kind="ExternalInput")
    t_lnb = nc.dram_tensor("lnb", [Ll, D_H], f32, kind="ExternalInput")
    t_c1w = nc.dram_tensor("c1w", [2, 128, D_H], f32, kind="ExternalInput")
    t_c1b = nc.dram_tensor("c1b", [1, D_H], f32, kind="ExternalInput")
    t_c2w = nc.dram_tensor("c2w", [2, 128, D_H // 2], f32, kind="ExternalInput")
    t_c2b = nc.dram_tensor("c2b", [1, D_H // 2], f32, kind="ExternalInput")
    t_c3w = nc.dram_tensor("c3w", [128, 1], f32, kind="ExternalInput")
    t_c3b = nc.dram_tensor("c3b", [1, 1], f32, kind="ExternalInput")
    t_y = nc.dram_tensor("y", [GPC, 1], f32, kind="ExternalOutput")

    def bcast_row(ap_2d, parts=128):
        # [1, D] dram AP -> [parts, D] with 0 partition stride
        return bass.AP(tensor=ap_2d.tensor, offset=ap_2d.offset,
                       ap=[[0, parts]] + list(ap_2d.ap[1:]))

    with tile.TileContext(nc) as tc, ExitStack() as ctx:
        const = ctx.enter_context(tc.tile_pool(name="const", bufs=1))
        resident = ctx.enter_context(tc.tile_pool(name="res", bufs=1))
        work = ctx.enter_context(tc.tile_pool(name="work", bufs=2))
        vpool = ctx.enter_context(tc.tile_pool(name="vpool", bufs=3))
        ohpool = ctx.enter_context(tc.tile_pool(name="ohpool", bufs=2))
        ipool = ctx.enter_context(tc.tile_pool(name="ipool", bufs=2))
        psA = ctx.enter_context(tc.tile_pool(name="psA", bufs=3, space="PSUM"))
        psO = ctx.enter_context(tc.tile_pool(name="psO", bufs=2, space="PSUM"))
        psT = ctx.enter_context(tc.tile_pool(name="psT", bufs=1, space="PSUM"))
        psP = ctx.enter_context(tc.tile_pool(name="psP", bufs=1, space="PSUM"))
        dram = ctx.enter_context(tc.tile_pool(name="dram", bufs=1, space="DRAM"))

        # ---- constants / resident tiles ----------------------------------
        ident32 = const.tile([128, 128], f32)
        make_identity(nc, ident32)
        iota_t = const.tile([128, 128], f16)
        nc.gpsimd.iota(iota_t[:], pattern=[[1, 128]], base=0,
                       channel_multiplier=0,
                       allow_small_or_imprecise_dtypes=True)

        dst_res = resident.tile([128, TOTCH], f16)
        nc.sync.dma_start(out=dst_res, in_=t_dst[:, :])
        dinvp_res = resident.tile([128, NB], f32)
        nc.sync.dma_start(out=dinvp_res, in_=t_dinvp[:, :])
        dselfp_res = resident.tile([128, NB], f32)
        nc.sync.dma_start(out=dselfp_res, in_=t_dselfp[:, :])
        dinvr_res = resident.tile([128, NB], f32)
        nc.sync.dma_start(out=dinvr_res, in_=t_dinvr[:, :])
        wpool_res = resident.tile([128, NB, GPC], f16)
        nc.sync.dma_start(out=wpool_res, in_=t_wpool[:, :, :])
        w1p_res = resident.tile([128, Ll, 2, D_H], f16)
        nc.sync.dma_start(out=w1p_res,
                          in_=t_w1p.rearrange("l k p d -> p l k d"))
        w2p_res = resident.tile([128, Ll, 2, D_H], f16)
        nc.sync.dma_start(out=w2p_res,
                          in_=t_w2p.rearrange("l k p d -> p l k d"))
        linw_res = resident.tile([128, KT, D_H], f16)
        nc.sync.dma_start(out=linw_res, in_=t_linw.rearrange("k p d -> p k d"))
        linb_res = resident.tile([128, D_H], f32)
        nc.gpsimd.dma_start(out=linb_res, in_=bcast_row(t_linb[:, :]))
        lng_res = resident.tile([128, Ll, D_H], f16)
        lnb_res = resident.tile([128, Ll, D_H], f16)
        for l in range(Ll):
            nc.gpsimd.dma_start(out=lng_res[:, l, :], in_=bcast_row(t_lng[l:l + 1, :]))
            nc.gpsimd.dma_start(out=lnb_res[:, l, :], in_=bcast_row(t_lnb[l:l + 1, :]))
        eps_t = const.tile([128, 1], f32)
        nc.vector.memset(eps_t, LN_EPS)

        # DRAM intermediates (fp8 gather tables, split in slot halves so the
        # lo AllGather hides under block compute mid-layer)
        HA = NP // 2
        ag_in = [dram.tile([NP, D_H], f8, name=f"ag_in_{l}") for l in range(Ll)]
        tab_lo = [dram.tile([NCORES * HA, D_H], f8, addr_space="Shared",
                            name=f"tablo_{l}") for l in range(Ll)]
        tab_hi = [dram.tile([NCORES * HA, D_H], f8, addr_space="Shared",
                            name=f"tabhi_{l}") for l in range(Ll)]

        # SBUF residents: h0^T (lhsT for the W2'' term) and hs = dinv*h of the
        # current layer (diagonal/self-loop term), avoiding DRAM round-trips
        h0T_res = resident.tile([128, NB, 2, 128], f16)
        hs_res = resident.tile([128, NB, D_H], f8)

        # ---- input layer: h0 = relu(x @ linw + b), write hs0 = dinv*h0 ---
        for b in range(NB):
            xt = work.tile([128, KT, 128], f16, tag="xstage")
            nc.sync.dma_start(out=xt, in_=t_xT[:, :, b * 128:(b + 1) * 128]
                              .rearrange("k p n -> p k n"))
            ps = psO.tile([128, D_H], f32, tag="outp")
            for k in range(KT):
                nc.tensor.matmul(ps[:], lhsT=xt[:, k, :], rhs=linw_res[:, k, :],
                                 start=(k == 0), stop=(k == KT - 1))
            # relu(ps + bias) : add bias on DVE, relu on ACT (fp32 for transpose)
            tmp = work.tile([128, D_H], f32, tag="lntmp")
            nc.vector.tensor_add(out=tmp[:], in0=ps[:], in1=linb_res[:])
            h0 = work.tile([128, D_H], f32, tag="h0f")
            nc.scalar.activation(out=h0[:], in_=tmp[:],
                                 func=mybir.ActivationFunctionType.Relu)
            # h0T -> resident SBUF (lhsT for the W2'' term every layer)
            trp = psT.tile([128, 256], f32, tag="trp")
            nc.tensor.transpose(out=trp[:, 0:128], in_=h0[:, 0:128], identity=ident32[:])
            nc.tensor.transpose(out=trp[:, 128:256], in_=h0[:, 128:256], identity=ident32[:])
            nc.vector.tensor_copy(out=h0T_res[:, b, 0, :], in_=trp[:, 0:128])
            nc.vector.tensor_copy(out=h0T_res[:, b, 1, :], in_=trp[:, 128:256])
            # hs0 = dinv * h0 -> resident + ag_in[0]
            nc.scalar.activation(out=hs_res[:, b, :], in_=h0[:],
                                 func=mybir.ActivationFunctionType.Identity,
                                 scale=dinvr_res[:, b:b + 1])
            nc.sync.dma_start(out=ag_in[0][b * 128:(b + 1) * 128, :],
                              in_=hs_res[:, b, :])

        pool_ps = psP.tile([GPC, D_H], f32)
        gq = [0]  # swdge queue rotation counter

        # ---- layers -------------------------------------------------------
        for l in range(Ll):
            if "nocc" not in dbg:
                for lo_, hi_, tabt in ((0, HA, tab_lo), (HA, NP, tab_hi)):
                    nc.gpsimd.collective_compute(
                        "AllGather", mybir.AluOpType.bypass,
                        ins=[ag_in[l][lo_:hi_]], outs=[tabt[l][:]],
                        replica_groups=[list(range(NCORES))],
                    )

            for r in range(NR):
                # gather stage for this round: one dma_gather per group
                vt = {}
                for g in range(NGROUP):
                    seg = int(idx_seg_len[r, g])
                    if seg == 0:
                        continue
                    nidx = seg * 128
                    it = ipool.tile([128, CSEG_MAX * 8], i16, tag=f"idx{g}")
                    c0 = int(idx_seg_off[r, g]) * 8
                    nc.sync.dma_start(out=it[:, :seg * 8],
                                      in_=t_idx[:, c0:c0 + seg * 8])
                    v = vpool.tile([128, CSEG_MAX, D_H], f8, tag=f"v{g}")
                    if "nogather" in dbg:
                        nc.vector.memset(v[:, :seg, :], 0.25)
                    else:
                        tab = tab_lo[l] if g < 2 else tab_hi[l]
                        goff = (g % 2) * GROUP_ROWS
                        for s0 in range(0, seg, 8):
                            sub = min(8, seg - s0)
                            nc.gpsimd.dma_gather(
                                v[:, s0:s0 + sub, :],
                                tab[goff:goff + GROUP_ROWS, :],
                                it[:, s0 * 8:(s0 + sub) * 8],
                                num_idxs=sub * 128,
                                num_idxs_reg=sub * 128,
                                elem_size=D_H,
                                queue_num=gq[0] % 4,
                            )
                            gq[0] += 1
                    vt[g] = v

                for b in range(4 * r, 4 * r + 4):
                    cb = int(C_b[b])
                    # one-hot for the whole block: [128, cb, 128] fp16
                    oh = ohpool.tile([128, CB_MAX, 128], f8, tag="oh")
                    d0 = int(dst_cols_off[b, 0])
                    dst_sl = dst_res[:, d0:d0 + cb]
                    dst_b = bass.AP(tensor=dst_sl.tensor, offset=dst_sl.offset,
                                    ap=[dst_sl.ap[0], dst_sl.ap[1], [0, 128]])
                    io_sl = iota_t[:, :]
                    iota_b = bass.AP(tensor=io_sl.tensor, offset=io_sl.offset,
                                     ap=[io_sl.ap[0], [0, cb], io_sl.ap[1]])
                    if "nooh" in dbg:
                        nc.vector.memset(oh[:, :cb, :], 0.0)
                    else:
                        nc.vector.tensor_tensor(out=oh[:, :cb, :], in0=dst_b,
                                                in1=iota_b, op=mybir.AluOpType.is_equal)

                    # segment-sum into psum
                    aps = psA.tile([128, D_H], f32, tag="acc")
                    if "noseg" in dbg:
                        nc.vector.memset(aps[:], 0.125)
                    else:
                        mm = 0
                        for g in range(NGROUP):
                            cbg = int(C_bg[b, g])
                            for j in range(cbg):
                                ohcol = int(dst_cols_off[b, g]) - d0 + j
                                vslot = int(vslot_off[b, g]) + j
                                nc.tensor.matmul(
                                    aps[:], lhsT=oh[:, ohcol, :],
                                    rhs=vt[g][:, vslot, :],
                                    start=(mm == 0), stop=(mm == cb - 1),
                                )
                                mm += 1
                    # a = (1-alpha)*dinv_dst * psum  (fp32, on ACT)
                    a_sb = work.tile([128, D_H], f32, tag="asb")
                    nc.scalar.activation(out=a_sb[:], in_=aps[:],
                                         func=mybir.ActivationFunctionType.Identity,
                                         scale=dinvp_res[:, b:b + 1])
                    # + diagonal (self-loop) term: dselfp * hs_prev (resident)
                    a_sb2 = work.tile([128, D_H], f32, tag="asb2")
                    nc.vector.scalar_tensor_tensor(
                        out=a_sb2[:], in0=hs_res[:, b, :],
                        scalar=dselfp_res[:, b:b + 1], in1=a_sb[:],
                        op0=mybir.AluOpType.mult, op1=mybir.AluOpType.add)
                    # aT via PE transpose (fp32 psum; fp16 psum reads are slow)
                    trp = psT.tile([128, 256], f32, tag="trp")
                    nc.tensor.transpose(out=trp[:, 0:128], in_=a_sb2[:, 0:128],
                                        identity=ident32[:])
                    nc.tensor.transpose(out=trp[:, 128:256], in_=a_sb2[:, 128:256],
                                        identity=ident32[:])
                    aT = work.tile([128, 2, 128], f16, tag="aT")
                    nc.vector.tensor_copy(out=aT[:, 0, :], in_=trp[:, 0:128])
                    nc.vector.tensor_copy(out=aT[:, 1, :], in_=trp[:, 128:256])

                    # out = a @ W1' + h0 @ W2''
                    ops = psO.tile([128, D_H], f32, tag="outp")
                    nc.tensor.matmul(ops[:], lhsT=aT[:, 0, :],
                                     rhs=w1p_res[:, l, 0, :], start=True, stop=False)
                    nc.tensor.matmul(ops[:], lhsT=aT[:, 1, :],
                                     rhs=w1p_res[:, l, 1, :], start=False, stop=False)
                    nc.tensor.matmul(ops[:], lhsT=h0T_res[:, b, 0, :],
                                     rhs=w2p_res[:, l, 0, :], start=False, stop=False)
                    nc.tensor.matmul(ops[:], lhsT=h0T_res[:, b, 1, :],
                                     rhs=w2p_res[:, l, 1, :], start=False, stop=True)

                    # LayerNorm + gamma/beta + relu
                    stats = work.tile([128, 6], f32, tag="stats")
                    nc.vector.bn_stats(out=stats[:], in_=ops[:])
                    mv = work.tile([128, 2], f32, tag="mv")
                    nc.vector.bn_aggr(out=mv[:], in_=stats[:])
                    rstd = work.tile([128, 1], f32, tag="rstd")
                    nc.scalar.activation(out=rstd[:], in_=mv[:, 1:2],
                                         func=mybir.ActivationFunctionType.Abs_reciprocal_sqrt,
                                         bias=eps_t[:], scale=1.0)
                    nmr = work.tile([128, 1], f32, tag="nmr")
                    nc.vector.tensor_scalar(out=nmr[:], in0=mv[:, 0:1],
                                            scalar1=rstd[:], scalar2=-1.0,
                                            op0=mybir.AluOpType.mult,
                                            op1=mybir.AluOpType.mult)
                    normed = work.tile([128, D_H], f16, tag="normed")
                    nc.scalar.activation(out=normed[:], in_=ops[:],
                                         func=mybir.ActivationFunctionType.Identity,
                                         bias=nmr[:], scale=rstd[:])
                    # gamma * normed + beta, then relu
                    gb = work.tile([128, D_H], f16, tag="gb")
                    nc.vector.scalar_tensor_tensor(
                        out=gb[:], in0=normed[:], scalar=1.0,
                        in1=lng_res[:, l, :],
                        op0=mybir.AluOpType.mult, op1=mybir.AluOpType.mult)
                    hn = work.tile([128, D_H], f16, tag="hn")
                    nc.vector.tensor_tensor(out=hn[:], in0=gb[:],
                                            in1=lnb_res[:, l, :],
                                            op=mybir.AluOpType.add)
                    hr = work.tile([128, D_H], f16, tag="hr")
                    nc.scalar.activation(out=hr[:], in_=hn[:],
                                         func=mybir.ActivationFunctionType.Relu)

                    if l == Ll - 1:
                        if "nopool" not in dbg:
                            nc.tensor.matmul(pool_ps[:], lhsT=wpool_res[:, b, :],
                                             rhs=hr[:], start=(b == 0),
                                             stop=(b == NB - 1),
                                             skip_group_check=True)
                    else:
                        nc.scalar.activation(out=hs_res[:, b, :], in_=hr[:],
                                             func=mybir.ActivationFunctionType.Identity,
                                             scale=dinvr_res[:, b:b + 1])
                        nc.sync.dma_start(
                            out=ag_in[l + 1][b * 128:(b + 1) * 128, :],
                            in_=hs_res[:, b, :])

        # ---- head ---------------------------------------------------------
        pooled = work.tile([GPC, D_H], f32, tag="pooled")
        if "nopool" in dbg:
            nc.vector.memset(pooled[:], 0.5)
        else:
            nc.vector.tensor_copy(out=pooled[:], in_=pool_ps[:])

        def head_mm(z, kdim, wtile_list, btile, relu, outdim):
            # z: [GPC, kdim] fp32 sbuf -> out [GPC, outdim] fp32
            trp2 = psT.tile([128, 256], f32, tag="trp")
            zT = work.tile([128, (kdim + 127) // 128, GPC], f32, tag="zT")
            for k in range((kdim + 127) // 128):
                kk = min(128, kdim - k * 128)
                nc.tensor.transpose(out=trp2[:kk, k * 128:k * 128 + GPC],
                                    in_=z[:, k * 128:k * 128 + kk],
                                    identity=ident32[:GPC, :GPC])
                nc.vector.tensor_copy(out=zT[:kk, k, :],
                                      in_=trp2[:kk, k * 128:k * 128 + GPC])
            ps = psO.tile([GPC, max(outdim, 1)], f32, tag="headp", bufs=1)
            nk = (kdim + 127) // 128
            for k in range(nk):
                kk = min(128, kdim - k * 128)
                nc.tensor.matmul(ps[:], lhsT=zT[:kk, k, :], rhs=wtile_list[k][:kk, :],
                                 start=(k == 0), stop=(k == nk - 1))
            zo = work.tile([GPC, outdim], f32, tag="zo")
            nc.vector.tensor_add(out=zo[:], in0=ps[:], in1=btile[:GPC, :])
            if relu:
                zr = work.tile([GPC, outdim], f32, tag="zr")
                nc.scalar.activation(out=zr[:], in_=zo[:],
                                     func=mybir.ActivationFunctionType.Relu)
                return zr
            return zo

        c1w_t = resident.tile([128, 2, D_H], f32)
        nc.sync.dma_start(out=c1w_t, in_=t_c1w.rearrange("k p d -> p k d"))
        c1b_t = resident.tile([128, D_H], f32)
        nc.gpsimd.dma_start(out=c1b_t, in_=bcast_row(t_c1b[:, :]))
        c2w_t = resident.tile([128, 2, D_H // 2], f32)
        nc.sync.dma_start(out=c2w_t, in_=t_c2w.rearrange("k p d -> p k d"))
        c2b_t = resident.tile([128, D_H // 2], f32)
        nc.gpsimd.dma_start(out=c2b_t, in_=bcast_row(t_c2b[:, :]))
        c3w_t = resident.tile([128, 1], f32)
        nc.sync.dma_start(out=c3w_t, in_=t_c3w[:, :])
        c3b_t = resident.tile([128, 1], f32)
        nc.gpsimd.dma_start(out=c3b_t, in_=bcast_row(t_c3b[:, :]))

        z1 = head_mm(pooled, D_H, [c1w_t[:, 0, :], c1w_t[:, 1, :]], c1b_t, True, D_H)
        z2 = head_mm(z1, D_H, [c2w_t[:, 0, :], c2w_t[:, 1, :]], c2b_t, True, D_H // 2)
        z3 = head_mm(z2, D_H // 2, [c3w_t], c3b_t, False, 1)
        nc.sync.dma_start(out=t_y[:, :], in_=z3[:])

    nc.compile()
    return nc


# ---------------------------------------------------------------------------
# entry point
# ---------------------------------------------------------------------------

_CACHE = {}


def kernel(**inputs):
    global LAST_EXEC_NS
    trace = bool(os.environ.get("BASS_TRACE"))
    if trace:
        _maybe_register_ntff_hook()

    in_maps, meta = preprocess(**inputs)
    ckey = ("k", meta["NP"], meta["TOTCH"])
    if ckey not in _CACHE:
        _CACHE[ckey] = build_kernel(meta)
    nc = _CACHE[ckey]

    res = run_bass_kernel_spmd(nc, in_maps, core_ids=list(range(NCORES)),
                               trace=trace)
    LAST_EXEC_NS = res.exec_time_ns
    ycat = np.concatenate([res.results[c]["y"].reshape(-1)
                           for c in range(NCORES)])
    y = np.empty(G, np.float32)
    y[meta["graph_order"]] = ycat
    return y

